# revision 38
# baseline (speedup 1.0000x reference)
"""Trainium2 Bass kernel for octonion causal self-attention (fp8/bf16 hybrid).

Sharding: 8 cores = 4 batches x 2 head-groups. Core c handles batch b=c//2 and
head-group g=c%2 (octonion output components 4g..4g+3 = heads 8g..8g+7).
Host sums the two wo-partials per batch and transposes. No collectives.

Precision scheme (tolerance 2e-2 relative to global max; this lands ~7.5e-3):
- Ternary weights are exactly representable in fp8e4 (+-1/0 for q,k,wo;
  +-0.25 for v); per-matrix ternary scales are averaged to one scalar per
  tensor (~0.3% spread) and folded into the exp() scale and the final output
  scale.
- Q/K projections: x quantized to fp8e4, DoubleRow matmuls (2 rows/cycle).
  The x-quantization error only perturbs attention scores (benign).
- V projection: DoubleRow over BOTH x8 and the fp8-quantized residual
  (x - x8) -> bf16-level accuracy at fp8-DR speed. v is stored bf16.
- Attention computed transposed (S^T: k-tile stationary, q moving): exp()
  writes PV-ready bf16 P~^T tiles directly - no PE transposes, no copy storm.
  Softmax denominators via ones-stationary matmul; 1/l broadcast across
  partitions with a rank-1 PE matmul; normalization fused into the y copy.
- mixer in bf16; wo keeps exact fp8 ternary weights with bf16 moving z.
- RoPE uses the evens-first output permutation folded into the q/k weight
  layout; the partner swap is two 64-partition SBUF-SBUF DMA copies.
"""

import math
import os
from contextlib import ExitStack

import numpy as np

B, T, C, H, D = 4, 1024, 2048, 16, 128
C8 = C // 8  # 256
NCORES = 8
P = 128
NEG = -1.0e30


# ---------------- octonion tables (matches reference) ----------------
def _cd_conj(a):
    n = a.shape[0]
    if n == 1:
        return a
    h = n // 2
    return np.concatenate([_cd_conj(a[:h]), -a[h:]])


def _cd_mul(a, b):
    n = a.shape[0]
    if n == 1:
        return a * b
    h = n // 2
    a1, a2 = a[:h], a[h:]
    c1, c2 = b[:h], b[h:]
    return np.concatenate(
        [
            _cd_mul(a1, c1) - _cd_mul(_cd_conj(c2), a2),
            _cd_mul(c2, a1) + _cd_mul(a2, _cd_conj(c1)),
        ]
    )


def _octonion_tables():
    signs = np.zeros((8, 8), dtype=np.float32)
    widx = np.zeros((8, 8), dtype=np.int32)
    for i in range(8):
        for j in range(8):
            ei = np.zeros(8)
            ei[i] = 1.0
            ej = np.zeros(8)
            ej[j] = 1.0
            p = _cd_mul(ei, ej)
            k = int(np.argmax(np.abs(p)))
            signs[i, j] = np.sign(p[k])
            widx[i, j] = k
    return signs, widx


SIGNS, WIDX = _octonion_tables()


def _ternary_quantize(W: np.ndarray) -> np.ndarray:
    """Replicates reference ternary_ste forward pass bit-exactly (jnp on CPU)."""
    import jax
    import jax.numpy as jnp

    with jax.default_device(jax.devices("cpu")[0]):
        Wj = jnp.asarray(W)
        s = jnp.mean(jnp.abs(Wj), axis=(-2, -1), keepdims=True) + 1e-8
        Wq = jnp.clip(jnp.round(Wj / s), -1.0, 1.0) * s
        return np.asarray(Wq)


def _tern_split(W: np.ndarray, Wq: np.ndarray):
    """-> (T int {-1,0,1} [8,256,256] f32, s [8] f32, sbar f32)."""
    s = (np.abs(W.astype(np.float32)).mean(axis=(1, 2)) + np.float32(1e-8)).astype(
        np.float32
    )
    Tm = np.rint(Wq / s[:, None, None]).astype(np.float32)
    return Tm, s, np.float32(s.mean())


def _signed_full_T(Tm: np.ndarray, i: int) -> np.ndarray:
    """[2048, 256] block column for octonion output component i:
    rows j*256:(j+1)*256 = SIGNS[i,j] * Tm[i^j] (values in {-1,0,1})."""
    out = np.empty((C, C8), dtype=np.float32)
    for j in range(8):
        out[j * C8 : (j + 1) * C8, :] = SIGNS[i, j] * Tm[i ^ j]
    return out


_EVENS_FIRST = np.concatenate([np.arange(0, D, 2), np.arange(1, D, 2)])


def _fp8np():
    from concourse import mybir

    return mybir.dt.np(mybir.dt.float8e4)


def _fp8(a: np.ndarray) -> np.ndarray:
    return np.clip(a, -240.0, 240.0).astype(_fp8np())


def _prep_core_inputs(inputs: dict, b: int, g: int, wq_q, wk_q, wv_q, wo_q):
    from concourse import mybir

    x = inputs["x"]
    fc, fs = inputs["freqs_cos"], inputs["freqs_sin"]
    mixer_W, mixer_beta = inputs["mixer_W"], inputs["mixer_beta"]

    T_q, _, sq = _tern_split(np.asarray(inputs["wq"], np.float32), wq_q)
    T_k, _, sk = _tern_split(np.asarray(inputs["wk"], np.float32), wk_q)
    T_v, _, sv = _tern_split(np.asarray(inputs["wv"], np.float32), wv_q)
    T_o, _, so = _tern_split(np.asarray(inputs["wo"], np.float32), wo_q)

    m = {}
    # x transposed fp8 + fp8 residual: [p, ct, t]
    xT = np.ascontiguousarray(x[b].T).reshape(16, P, T).transpose(1, 0, 2)
    x8 = _fp8(xT)
    m["x8"] = x8
    m["dx8"] = _fp8(xT - x8.astype(np.float32))

    # q/k weights fp8 (+-1/0): [qk, li, dh, c_p, ct, d], evens-first outputs
    wqk8 = np.empty((2, 4, 2, P, 16, P), dtype=np.float32)
    for qk, Tm in enumerate((T_q, T_k)):
        for li in range(4):
            Bf = _signed_full_T(Tm, 4 * g + li)  # [2048, 256]
            for dh in range(2):
                Bh = Bf[:, dh * D : (dh + 1) * D][:, _EVENS_FIRST]
                wqk8[qk, li, dh] = Bh.reshape(16, P, P).transpose(1, 0, 2)
    m["wqk8"] = _fp8(wqk8)

    # v weights fp8 (+-0.25): [lp, c_p, ct, vd]
    wv8 = np.empty((2, P, 16, 512), dtype=np.float32)
    for lp in range(2):
        B2 = np.concatenate(
            [_signed_full_T(T_v, 4 * g + 2 * lp + u) for u in range(2)], axis=1
        )  # [2048, 512]
        wv8[lp] = 0.25 * B2.reshape(16, P, 512).transpose(1, 0, 2)
    m["wv8"] = _fp8(wv8)

    # wo fp8 (+-1): [ft, d_p, i, f]; i = local z tile (4 comps x 2 halves)
    wo8 = np.empty((16, P, 8, P), dtype=np.float32)
    for ft in range(16):
        i_o, fh = ft // 2, ft % 2
        for i in range(8):
            j = 4 * g + i // 2
            blk = SIGNS[i_o, j] * T_o[i_o ^ j]  # [256, 256]
            wo8[ft, :, i, :] = blk[
                (i % 2) * P : (i % 2 + 1) * P, fh * P : (fh + 1) * P
            ]
    m["wo8"] = _fp8(wo8)

    # mixer bf16: [i, d_p, j, e] = SIGNS*W[i^j]*beta
    bf = mybir.dt.np(mybir.dt.bfloat16)
    wmb = np.empty((8, P, 8, P), dtype=np.float32)
    for i in range(8):
        for j in range(8):
            wmb[i, :, j, :] = SIGNS[i, j] * mixer_W[i ^ j] * mixer_beta[None, :]
    m["wmb"] = wmb.astype(bf)

    # RoPE tables, evens-first layout: rows 0..63 even dims, 64..127 odd dims.
    # rope(q')[p] = q'[p]*cosd[p] + q'[p xor 64]*sind[p]
    cosP = np.ascontiguousarray(fc.T).astype(np.float32)  # [64, 1024]
    sinP = np.ascontiguousarray(fs.T).astype(np.float32)
    m["cosd"] = np.concatenate([cosP, cosP], axis=0).astype(bf)
    m["sind"] = np.concatenate([-sinP, sinP], axis=0).astype(bf)

    # causal triangle mask for diagonal 128-blocks: tri[k_p, q] = 0 if q>=k
    pidx = np.arange(P)
    m["tri"] = np.where(
        pidx[None, :] >= pidx[:, None], 0.0, NEG
    ).astype(np.float32)

    # folded scales per partition: col 0 = exp scale, col 1 = output scale
    cexp = np.float32(sq * sk / math.sqrt(D))
    cout = np.float32(4.0 * so * sv)
    scal = np.empty((P, 2), dtype=np.float32)
    scal[:, 0] = cexp
    scal[:, 1] = cout
    m["scal"] = scal
    return m


# ---------------- device program ----------------
_NC_CACHE = {}


def _build_nc(repeat: int = 1, split_dma: bool = True, deep: int = 4):
    key = (repeat, split_dma, deep)
    if key in _NC_CACHE:
        return _NC_CACHE[key]

    import concourse.mybir as mybir
    import concourse.tile as tile
    from concourse import bacc

    dt = mybir.dt
    ALU = mybir.AluOpType
    AF = mybir.ActivationFunctionType
    f32, bf16, f8 = dt.float32, dt.bfloat16, dt.float8e4
    DR = mybir.MatmulPerfMode.DoubleRow

    nc = bacc.Bacc("TRN2", target_bir_lowering=False)

    x8p = nc.declare_dram_parameter("x8", [P, 16, T], f8, isOutput=False)
    dx8p = nc.declare_dram_parameter("dx8", [P, 16, T], f8, isOutput=False)
    wqkp = nc.declare_dram_parameter("wqk8", [2, 4, 2, P, 16, P], f8, isOutput=False)
    wvp = nc.declare_dram_parameter("wv8", [2, P, 16, 512], f8, isOutput=False)
    wop = nc.declare_dram_parameter("wo8", [16, P, 8, P], f8, isOutput=False)
    wmp = nc.declare_dram_parameter("wmb", [8, P, 8, P], bf16, isOutput=False)
    cosp = nc.declare_dram_parameter("cosd", [P, T], bf16, isOutput=False)
    sinp = nc.declare_dram_parameter("sind", [P, T], bf16, isOutput=False)
    trip = nc.declare_dram_parameter("tri", [P, P], f32, isOutput=False)
    scalp = nc.declare_dram_parameter("scal", [P, 2], f32, isOutput=False)
    outT = nc.declare_dram_parameter("outT", [C, T], f32, isOutput=True)

    def _eng(idx):
        if not split_dma:
            return nc.sync
        return (nc.sync, nc.gpsimd, nc.scalar)[idx % 3]

    with tile.TileContext(nc) as tc, ExitStack() as ctx:
        cst = ctx.enter_context(tc.tile_pool(name="cst", bufs=1))
        stage = ctx.enter_context(tc.tile_pool(name="stage", bufs=4))
        small = ctx.enter_context(tc.tile_pool(name="small", bufs=4))

        for _rep in range(repeat):
            per_cm = tc.tile_pool(name="perrep", bufs=1)
            per = per_cm.__enter__()

            # ---- constants + resident tensors ----
            cos_sb = cst.tile([P, T], bf16, tag="cos")
            sin_sb = cst.tile([P, T], bf16, tag="sin")
            tri_sb = cst.tile([P, P], f32, tag="tri")
            scal_sb = cst.tile([P, 2], f32, tag="scal")
            ones1 = cst.tile([1, P], bf16, tag="ones1")
            onescol = cst.tile([P, 1], bf16, tag="onescol")
            nc.gpsimd.memset(ones1[:], 1.0)
            nc.gpsimd.memset(onescol[:], 1.0)
            ceng = nc.gpsimd if split_dma else nc.sync
            ceng.dma_start(cos_sb[:], cosp[:])
            ceng.dma_start(sin_sb[:], sinp[:])
            ceng.dma_start(tri_sb[:], trip[:])
            ceng.dma_start(scal_sb[:], scalp[:])

            v_sb = per.tile([P, 8, 8 * P], bf16, tag="vsb", name="vsbt")
            qT_h = [
                per.tile([P, T], bf16, tag=f"qT{i}", name=f"qTh{i}") for i in range(8)
            ]
            kT_h = [
                per.tile([P, T], bf16, tag=f"kT{i}", name=f"kTh{i}") for i in range(8)
            ]

            # ======== projection phase ========
            proj_cm = tc.tile_pool(name="projp", bufs=1)
            proj = proj_cm.__enter__()
            wqk_cm = tc.tile_pool(name="wqkp", bufs=4)
            wqk_p = wqk_cm.__enter__()
            rope_cm = tc.tile_pool(name="ropep", bufs=2)
            rope_p = rope_cm.__enter__()
            psp_cm = tc.tile_pool(name="psp", bufs=4, space="PSUM")
            ps_p = psp_cm.__enter__()

            # x8 split across all three DMA queues so the PE can start ASAP
            x8 = proj.tile([P, 16, T], f8, tag="x8", name="x8t")
            dx8 = proj.tile([P, 16, T], f8, tag="dx8", name="dx8t")
            for h8 in range(4):
                _eng(h8).dma_start(
                    x8[:, 4 * h8 : 4 * h8 + 4, :], x8p[:, 4 * h8 : 4 * h8 + 4, :]
                )
            for h8 in range(4):
                nc.sync.dma_start(
                    dx8[:, 4 * h8 : 4 * h8 + 4, :], dx8p[:, 4 * h8 : 4 * h8 + 4, :]
                )
            wv_sb = [
                proj.tile([P, 16, 512], f8, tag=f"wv{lp}", name=f"wvsb{lp}")
                for lp in range(2)
            ]
            for lp in range(2):
                for h8 in range(2):
                    nc.sync.dma_start(
                        wv_sb[lp][:, 8 * h8 : 8 * h8 + 8, :],
                        wvp[lp, :, 8 * h8 : 8 * h8 + 8, :],
                    )

            # ---- Q/K projections first (attention can start early),
            #      q/k interleaved per (li, dh) so head h tiles finish together
            for li in range(4):
                for dh in range(2):
                    hh = li * 2 + dh
                    for qk, dest_h in ((0, qT_h), (1, kT_h)):
                        wt = wqk_p.tile([P, 16, P], f8, tag="wqk")
                        nc.sync.dma_start(wt[:], wqkp[qk, li, dh])
                        pps = [
                            ps_p.tile([P, 512], f32, tag="proj", name=f"pp{t}")
                            for t in range(2)
                        ]
                        for c in range(8):
                            for tci in range(2):
                                nc.tensor.matmul(
                                    pps[tci][:],
                                    wt[:, 2 * c : 2 * c + 2, :],
                                    x8[:, 2 * c : 2 * c + 2, tci * 512 : (tci + 1) * 512],
                                    start=(c == 0),
                                    stop=(c == 7),
                                    perf_mode=DR,
                                )
                        for tci in range(2):
                            tsl = slice(tci * 512, (tci + 1) * 512)
                            qsb = rope_p.tile([P, 512], bf16, tag="qsb")
                            nc.scalar.activation(qsb[:], pps[tci][:], AF.Copy)
                            qsw = rope_p.tile([P, 512], bf16, tag="qsw")
                            eng = (nc.gpsimd, nc.scalar)[(hh + tci) % 2]
                            eng.dma_start(qsw[0:64, :], qsb[64:128, :])
                            eng.dma_start(qsw[64:128, :], qsb[0:64, :])
                            t1 = rope_p.tile([P, 512], bf16, tag="t1")
                            nc.vector.tensor_tensor(
                                t1[:], qsb[:], cos_sb[:, tsl], ALU.mult
                            )
                            t2 = rope_p.tile([P, 512], bf16, tag="t2")
                            nc.vector.tensor_tensor(
                                t2[:], qsw[:], sin_sb[:, tsl], ALU.mult
                            )
                            nc.vector.tensor_tensor(
                                dest_h[hh][:, tsl], t1[:], t2[:], ALU.add
                            )

            # ---- V projection: DoubleRow over x8 and dx8 -> bf16 ----
            for tt in range(8):
                vps = [
                    ps_p.tile([P, 512], f32, tag="proj", name=f"vps{tt}_{lp}")
                    for lp in range(2)
                ]
                for c in range(8):
                    for lp in range(2):
                        nc.tensor.matmul(
                            vps[lp][:],
                            x8[:, 2 * c : 2 * c + 2, tt * P : (tt + 1) * P],
                            wv_sb[lp][:, 2 * c : 2 * c + 2, :],
                            start=(c == 0),
                            stop=False,
                            perf_mode=DR,
                        )
                for c in range(8):
                    for lp in range(2):
                        nc.tensor.matmul(
                            vps[lp][:],
                            dx8[:, 2 * c : 2 * c + 2, tt * P : (tt + 1) * P],
                            wv_sb[lp][:, 2 * c : 2 * c + 2, :],
                            start=False,
                            stop=(c == 7),
                            perf_mode=DR,
                        )
                for lp in range(2):
                    nc.any.tensor_copy(
                        out=v_sb[:, tt, lp * 512 : (lp + 1) * 512], in_=vps[lp][:]
                    )

            psp_cm.__exit__(None, None, None)
            rope_cm.__exit__(None, None, None)
            wqk_cm.__exit__(None, None, None)
            proj_cm.__exit__(None, None, None)

            # ======== attention + mixer + wo phase ========
            pt_cm = tc.tile_pool(name="ptp", bufs=5)
            pt_pool = pt_cm.__enter__()
            wm_cm = tc.tile_pool(name="wmp", bufs=8)
            wm_pool = wm_cm.__enter__()
            w2_cm = tc.tile_pool(name="w2p", bufs=16)
            w2_pool = w2_cm.__enter__()
            pss_cm = tc.tile_pool(name="pss", bufs=3, space="PSUM")
            ps_s = pss_cm.__enter__()
            psy_cm = tc.tile_pool(name="psy", bufs=3, space="PSUM")
            ps_y = psy_cm.__enter__()
            psl_cm = tc.tile_pool(name="psl", bufs=1, space="PSUM")
            ps_l = psl_cm.__enter__()
            psb_cm = tc.tile_pool(name="psb", bufs=1, space="PSUM")
            ps_b = psb_cm.__enter__()

            wm_sb = []
            for i in range(8):
                wmt = wm_pool.tile([P, 8, P], bf16, tag="wm", name=f"wm{i}")
                nc.sync.dma_start(wmt[:], wmp[i])
                wm_sb.append(wmt)
            wo_sb = []
            for ft in range(16):
                wot = w2_pool.tile([P, 8, P], f8, tag="wo", name=f"wo{ft}")
                nc.sync.dma_start(wot[:], wop[ft])
                wo_sb.append(wot)

            y8s = [
                per.tile([P, 8, 512], bf16, tag=f"y{qc}", name=f"ybf_{qc}")
                for qc in range(2)
            ]
            z8s = [
                per.tile([P, 8, 512], f8, tag=f"z{qc}", name=f"z8_{qc}")
                for qc in range(2)
            ]
            dz8s = [
                per.tile([P, 8, 512], f8, tag=f"dz{qc}", name=f"dz8_{qc}")
                for qc in range(2)
            ]

            # per-(qc,head) attention stages (3-deep software pipeline on PE)
            def stage_a(qc, h):
                """QK + mask + exp -> P~^T bf16 tile."""
                nkt = 4 * (qc + 1)
                q0 = qc * 512
                pt = pt_pool.tile([P, 8, 512], bf16, tag="PT", name=f"PT{qc}_{h}")
                for kt in range(nkt):
                    j = kt - 4 * qc
                    w0 = max(0, j) * P  # valid q starts here (chunk-local)
                    if w0 > 0:
                        nc.gpsimd.memset(pt[:, kt, 0:w0], 0.0)
                    sps = ps_s.tile(
                        [P, 512], f32, tag="S", name=f"sps{qc}_{h}_{kt}"
                    )
                    nc.tensor.matmul(
                        sps[:, w0:512],
                        kT_h[h][:, kt * P : (kt + 1) * P],
                        qT_h[h][:, q0 + w0 : q0 + 512],
                        start=True,
                        stop=True,
                    )
                    if j >= 0:
                        nc.vector.tensor_tensor(
                            sps[:, w0 : w0 + P],
                            sps[:, w0 : w0 + P],
                            tri_sb[:],
                            ALU.add,
                        )
                    nc.scalar.activation(
                        pt[:, kt, w0:512],
                        sps[:, w0:512],
                        AF.Exp,
                        scale=scal_sb[:, 0:1],
                    )
                return pt

            def stage_b(qc, h, pt):
                """softmax denominator l + bf16 reciprocal."""
                nkt = 4 * (qc + 1)
                lps = ps_l.tile([1, 512], f32, tag="l", name=f"l{qc}_{h}")
                for kt in range(nkt):
                    nc.tensor.matmul(
                        lps[:],
                        onescol[:],
                        pt[:, kt, :],
                        start=(kt == 0),
                        stop=(kt == nkt - 1),
                    )
                rec = small.tile([1, 512], bf16, tag="rec")
                with nc.allow_low_precision(
                    reason="1/l broadcast via bf16 rank-1 matmul; 0.4% rel"
                ):
                    nc.vector.reciprocal(rec[:], lps[:])
                return rec

            def stage_c(qc, h, pt, rec):
                """1/l broadcast, PV, normalization fused into the y copy."""
                nkt = 4 * (qc + 1)
                bps = ps_b.tile([P, 512], f32, tag="bc", name=f"bc{qc}_{h}")
                nc.tensor.matmul(bps[:], ones1[:], rec[:], start=True, stop=True)
                rbc = small.tile([P, 512], bf16, tag="rbc")
                nc.any.tensor_copy(out=rbc[:], in_=bps[:])
                yps = ps_y.tile([P, 512], f32, tag="y", name=f"yps{qc}_{h}")
                for kt in range(nkt):
                    nc.tensor.matmul(
                        yps[:],
                        v_sb[:, kt, h * P : (h + 1) * P],
                        pt[:, kt, :],
                        start=(kt == 0),
                        stop=(kt == nkt - 1),
                    )
                nc.vector.tensor_tensor(
                    y8s[qc][:, h, :], yps[:], rbc[:], ALU.mult
                )

            def mixer(qc, i):
                zps = ps_s.tile([P, 512], f32, tag="S", name=f"zps{qc}_{i}")
                for jj in range(8):
                    nc.tensor.matmul(
                        zps[:],
                        wm_sb[i][:, jj, :],
                        y8s[qc][:, jj, :],
                        start=(jj == 0),
                        stop=(jj == 7),
                    )
                # z in fp8 + fp8 residual so wo can run DoubleRow
                nc.any.tensor_copy(out=z8s[qc][:, i, :], in_=zps[:])
                nc.any.tensor_tensor(
                    dz8s[qc][:, i, :], zps[:], z8s[qc][:, i, :], ALU.subtract
                )

            def wo_ft(qc, ft):
                ops = ps_y.tile([P, 512], f32, tag="y", name=f"ops{qc}_{ft}")
                for c in range(4):
                    nc.tensor.matmul(
                        ops[:],
                        wo_sb[ft][:, 2 * c : 2 * c + 2, :],
                        z8s[qc][:, 2 * c : 2 * c + 2, :],
                        start=(c == 0),
                        stop=False,
                        perf_mode=DR,
                    )
                for c in range(4):
                    nc.tensor.matmul(
                        ops[:],
                        wo_sb[ft][:, 2 * c : 2 * c + 2, :],
                        dz8s[qc][:, 2 * c : 2 * c + 2, :],
                        start=False,
                        stop=(c == 3),
                        perf_mode=DR,
                    )
                osb = stage.tile([P, 512], f32, tag="osb")
                nc.any.tensor_scalar(
                    out=osb[:],
                    in0=ops[:],
                    scalar1=scal_sb[:, 1:2],
                    scalar2=None,
                    op0=ALU.mult,
                )
                nc.sync.dma_start(
                    outT[ft * P : (ft + 1) * P, qc * 512 : qc * 512 + 512], osb[:]
                )

            units = [(qc, h) for qc in range(2) for h in range(8)]
            arts = {}
            # software pipeline: A(i), B(i-db), C(i-dc) gives the exp chain
            # db full units of slack before the l matmuls need it.
            # mixer(qc0) after C(0,7); wo(qc0) after mixer(qc0).
            db, dc = (1, 2) if deep == 3 else (2, 3)
            mx0 = 8 + dc  # idx at which C(0,7) has been emitted
            inject = {
                mx0 + 1: lambda: [mixer(0, i) for i in range(4)],
                mx0 + 2: lambda: [mixer(0, i) for i in range(4, 8)],
            }
            nwo = 16 - (mx0 + 3)
            if nwo > 0:
                per_i = (16 + nwo - 1) // nwo
                for s, idx in enumerate(range(mx0 + 3, 16)):
                    lo_, hi_ = per_i * s, min(per_i * (s + 1), 16)
                    inject[idx] = (
                        lambda lo_=lo_, hi_=hi_: [
                            wo_ft(0, ft) for ft in range(lo_, hi_)
                        ]
                    )
            for idx, (qc, h) in enumerate(units):
                pt = stage_a(qc, h)
                arts[idx] = [pt, None]
                if idx >= db:
                    pqc, ph = units[idx - db]
                    arts[idx - db][1] = stage_b(pqc, ph, arts[idx - db][0])
                if idx >= dc:
                    pqc, ph = units[idx - dc]
                    stage_c(pqc, ph, *arts.pop(idx - dc))
                if idx in inject:
                    inject[idx]()
            for idx in range(16 - db, 16):
                arts[idx][1] = stage_b(*units[idx], arts[idx][0])
                done_c = idx - dc + db
                if 0 <= done_c < 16 and done_c in arts:
                    stage_c(*units[done_c], *arts.pop(done_c))
            for idx in sorted(arts):
                stage_c(*units[idx], *arts.pop(idx))
            for i in range(8):
                mixer(1, i)
            for ft in range(16):
                wo_ft(1, ft)

            psb_cm.__exit__(None, None, None)
            psl_cm.__exit__(None, None, None)
            psy_cm.__exit__(None, None, None)
            pss_cm.__exit__(None, None, None)
            w2_cm.__exit__(None, None, None)
            wm_cm.__exit__(None, None, None)
            pt_cm.__exit__(None, None, None)
            per_cm.__exit__(None, None, None)

    nc.finalize()
    _NC_CACHE[key] = nc
    return nc


def _run(inputs: dict, trace: bool = False):
    from concourse.bass_utils import run_bass_kernel_spmd

    wq_q = _ternary_quantize(np.asarray(inputs["wq"], dtype=np.float32))
    wk_q = _ternary_quantize(np.asarray(inputs["wk"], dtype=np.float32))
    wv_q = _ternary_quantize(np.asarray(inputs["wv"], dtype=np.float32))
    wo_q = _ternary_quantize(np.asarray(inputs["wo"], dtype=np.float32))

    in_maps = []
    for c in range(NCORES):
        b, g = c // 2, c % 2
        in_maps.append(_prep_core_inputs(inputs, b, g, wq_q, wk_q, wv_q, wo_q))

    nc = _build_nc()
    res = run_bass_kernel_spmd(nc, in_maps, list(range(NCORES)), trace=trace)

    out = np.empty((B, T, C), dtype=np.float32)
    for b in range(B):
        acc = res.results[2 * b]["outT"] + res.results[2 * b + 1]["outT"]
        out[b] = acc.T
    return out, res


def kernel(**inputs) -> np.ndarray:
    out, _ = _run(inputs, trace=False)
    return out
